# revision 5
# baseline (speedup 1.0000x reference)
"""MMD loss (RBF kernel, sigma=1) on 8 Trainium2 NeuronCores.

kernel(x, y): x, y float32 [20000, 64] -> float32 scalar
    kxx/nX^2 + kyy/nY^2 - 2*kxy/(nX*nY),  k** = sum_ij exp(-||a_i-b_j||^2/2)

Design (v2)
-----------
exp(-(|a|^2+|b|^2-2ab)/2) = exp(a.b + s_b) * e^{s_a}, s_v = -|v|^2/2.
One fp16 matmul with K=66 produces the PRE-SCALED exponent
    m' = A*(a.b + s_b),  A = 2^23/ln2,
via rows [sqrt(A)*a (64); 32768; 32] x cols [sqrt(A)*b (64); gh; gl]
where gh*32768 + gl*32 = A*s_b (+2^23 for doubled columns: exp(m+ln2)=2e^m).
The per-row factor e^{s_a} is applied on the HOST to the per-row partial
sums (pad rows get factor 0), which keeps K = 66 and removes kill codes.

Each PSUM chunk is consumed by one of two engines:
  * ScalarE (ACT): exp activation (scale=1/A) + accum_out row sums.
  * VectorE (DVE): Schraudolph fast-exp: TS1 = rint(max(m'+B',0)) -> int32,
    bitcast int32 -> fp32 IS ~exp(m) (B' = 127*2^23 - 482573 tuned for
    zero-mean relative error); TS2 passthrough + accum_out row sums.
Both engines run concurrently; the PE (pinned cold at 1.2 GHz on this
platform, 1 col/cycle) becomes the bottleneck at ~98% busy.

Sharding: row blocks of 2500 across 8 cores (SPMD identical program).
kxx/kyy use block symmetry: every core computes two triangle-skipped
windows (12500 + 10000 wide, all columns doubled) plus 40 in-tile
diagonal squares (coda, weight 1); cores 0-3 take the x-side 12500
window (distance 1-4) + y-side 10000 (distance 1-3), cores 4-7 swap
sides. Since kxx and kyy carry the same coefficient, the host just sums
both into one "same-side" accumulator. kxy: full 20000 y columns.
"""

import os

import numpy as np

# problem dims (hardcoded per contract)
N = 20000
D = 64
CORES = 8
BLOCK = N // CORES  # 2500
TILE = 128
N_TILES = 20  # ceil(2500/128)
PAD_BLOCK = TILE * N_TILES  # 2560
WIN1 = 5 * BLOCK  # 12500
WIN2 = 4 * BLOCK  # 10000
K = D + 2  # 66 contraction rows
CHUNK = 2048  # chunk width (4 PSUM banks)
MM_N = 512  # matmul moving free dim (1 PSUM bank fp32)

A_SCALE = float(2**23) / float(np.log(2.0))  # 12102203.161561485
A1 = float(np.sqrt(A_SCALE))
B_SHIFT = float(127 * 2**23 - 482573)  # Schraudolph, zero-mean C
DOUBLE = float(2**23)  # A*ln2 exactly: doubles the term
KILL_GH = -65504.0  # pad cols: gh*32768 = -2.1e9 -> exp -> 0

_CACHE: dict = {}


def _eq_chunks(total, chunk=CHUNK):
    """Equal-width EVEN chunks (each <= chunk); total must be even.
    Even widths keep the DVE decode pass in its 2x perf mode."""
    if total <= 0:
        return []
    assert total % 2 == 0
    n = -(-total // chunk)
    base2, rem2 = divmod(total // 2, n)
    out, pos = [], 0
    for i in range(n):
        w = 2 * (base2 + (1 if i < rem2 else 0))
        out.append((pos, w))
        pos += w
    return out


# items: (cols_name, rw_name, ncols, triangle?)
_ITEMS = [
    ("cols1", "rw1", WIN1, True),
    ("cols2", "rw2", WIN2, True),
    ("colsyf", "rwx", N, False),
]


def _chunk_schedule():
    """Main-loop chunks: list of (item_idx, r, c0, cn, use_dve)."""
    out = []
    gi = 0
    for it, (_c, _r, ncols, tri) in enumerate(_ITEMS):
        for r in range(N_TILES):
            base = TILE * (r + 1) if tri else 0
            for c0r, cn in _eq_chunks(ncols - base):
                use_dve = gi % 3 == 2
                out.append((it, r, base + c0r, cn, use_dve))
                gi += 1
    return out


def _slot_meta():
    """Per-slot (acc_class, side_token, r) for host reduction, in program
    order: 40 coda slots then the main-loop slots.

    side_token: which rw tensor's rows feed the slot's partitions:
      'x' -> rwx content, 'y' -> rwy2 content, '1' -> rw1, '2' -> rw2.
    acc_class: 0 = same-side (kxx+kyy), 1 = cross (kxy).
    """
    meta = []
    for r in range(N_TILES):
        meta.append((0, "x", r))
    for r in range(N_TILES):
        meta.append((0, "y", r))
    for it, r, _c0, _cn, _dve in _chunk_schedule():
        side = ("1", "2", "x")[it]
        acc = 1 if it == 2 else 0
        meta.append((acc, side, r))
    return meta


def _build_nc():
    import concourse.bacc as bacc
    import concourse.tile as tile
    from concourse import mybir

    f16 = mybir.dt.float16
    f32 = mybir.dt.float32
    i32 = mybir.dt.int32
    EXP = mybir.ActivationFunctionType.Exp
    ALU = mybir.AluOpType

    sched = _chunk_schedule()
    n_slots = 2 * N_TILES + len(sched)

    nc = bacc.Bacc("TRN2", target_bir_lowering=False)

    dram = {
        "cols1": nc.dram_tensor("cols1", [K, WIN1], f16, kind="ExternalInput"),
        "cols2": nc.dram_tensor("cols2", [K, WIN2], f16, kind="ExternalInput"),
        "colsyf": nc.dram_tensor("colsyf", [K, N], f16, kind="ExternalInput"),
        "colsqx": nc.dram_tensor("colsqx", [K, PAD_BLOCK], f16, kind="ExternalInput"),
        "colsqy": nc.dram_tensor("colsqy", [K, PAD_BLOCK], f16, kind="ExternalInput"),
        "rw1": nc.dram_tensor("rw1", [K, PAD_BLOCK], f16, kind="ExternalInput"),
        "rw2": nc.dram_tensor("rw2", [K, PAD_BLOCK], f16, kind="ExternalInput"),
        "rwx": nc.dram_tensor("rwx", [K, PAD_BLOCK], f16, kind="ExternalInput"),
        "rwy2": nc.dram_tensor("rwy2", [K, PAD_BLOCK], f16, kind="ExternalInput"),
    }
    parts_d = nc.dram_tensor("parts", [TILE, n_slots], f32, kind="ExternalOutput")

    with tile.TileContext(nc) as tc:
        with (
            tc.tile_pool(name="sb", bufs=1) as sb,
            tc.tile_pool(name="ps", bufs=2, space="PSUM") as ps,
        ):
            sbuf = {
                name: sb.tile([K, t.shape[1]], f16, name=f"sb_{name}")
                for name, t in dram.items()
            }
            parts = sb.tile([TILE, n_slots], f32)
            s_i32 = sb.tile([TILE, CHUNK], i32)
            s_dec = sb.tile([TILE, CHUNK], f32)
            zeros = sb.tile([TILE, 1], f32)
            nc.vector.memset(zeros, 0.0)
            nc.vector.memset(parts, 0.0)

            # DMA order: coda inputs first (they unblock the first chunks),
            # then rw1/rw2, then the column windows round-robin.
            nc.sync.dma_start(out=sbuf["colsqx"], in_=dram["colsqx"][:, :])
            nc.sync.dma_start(out=sbuf["rwx"], in_=dram["rwx"][:, :])
            nc.gpsimd.dma_start(out=sbuf["colsqy"], in_=dram["colsqy"][:, :])
            nc.gpsimd.dma_start(out=sbuf["rwy2"], in_=dram["rwy2"][:, :])
            nc.sync.dma_start(out=sbuf["rw1"], in_=dram["rw1"][:, :])
            nc.gpsimd.dma_start(out=sbuf["rw2"], in_=dram["rw2"][:, :])
            nc.sync.dma_start(out=sbuf["cols1"][:, :2048], in_=dram["cols1"][:, :2048])
            dma_engines = [nc.gpsimd, nc.sync]
            ei = 0
            rest = [("cols1", 2048, WIN1), ("cols2", 0, WIN2), ("colsyf", 0, N)]
            for name, start, total in rest:
                t = sbuf[name]
                left = total - start
                step = -(-left // 4)
                p0 = start
                while left > 0:
                    w = min(step, left)
                    dma_engines[ei % len(dma_engines)].dma_start(
                        out=t[:, p0 : p0 + w], in_=dram[name][:, p0 : p0 + w]
                    )
                    p0 += w
                    left -= w
                    ei += 1

            slot = 0

            def act_chunk(pt, c0, cn, slot):
                nc.scalar.activation(
                    out=pt[:, c0 : c0 + cn],
                    in_=pt[:, c0 : c0 + cn],
                    func=EXP,
                    bias=zeros[:, 0:1],
                    scale=1.0 / A_SCALE,
                    accum_out=parts[:, slot : slot + 1],
                )

            def dve_chunk(pt, cn, slot):
                nc.vector.tensor_scalar(
                    out=s_i32[:, :cn],
                    in0=pt[:, :cn],
                    scalar1=B_SHIFT,
                    scalar2=0.0,
                    op0=ALU.add,
                    op1=ALU.max,
                )
                nc.vector.tensor_scalar(
                    out=s_dec[:, :cn],
                    in0=s_i32[:, :cn].bitcast(f32),
                    scalar1=1.0,
                    scalar2=0.0,
                    op0=ALU.mult,
                    op1=ALU.add,
                    accum_out=parts[:, slot : slot + 1],
                )

            # --- coda: 40 in-tile diagonal squares, one slot each ---
            coda = [("rwx", "colsqx", r) for r in range(N_TILES)] + [
                ("rwy2", "colsqy", r) for r in range(N_TILES)
            ]
            for g0 in range(0, 40, 16):
                grp = coda[g0 : g0 + 16]
                pt = ps.tile([TILE, CHUNK], f32, tag="pt", name=f"ptc{g0}")
                for k, (rwn, cqn, r) in enumerate(grp):
                    sl = slice(TILE * r, TILE * (r + 1))
                    nc.tensor.matmul(
                        pt[:, TILE * k : TILE * (k + 1)],
                        sbuf[rwn][:, sl],
                        sbuf[cqn][:, sl],
                        start=True,
                        stop=True,
                    )
                for k in range(len(grp)):
                    act_chunk(pt, TILE * k, TILE, slot)
                    slot += 1

            # --- main items ---
            for it, r, c0, cn, use_dve in _chunk_schedule():
                cols_name, rw_name, _ncols, _tri = _ITEMS[it]
                cols, rw = sbuf[cols_name], sbuf[rw_name]
                lhsT = rw[:, r * TILE : (r + 1) * TILE]
                pt = ps.tile([TILE, CHUNK], f32, tag="pt", name=f"pt{slot}")
                for s0 in range(0, cn, MM_N):
                    sn = min(MM_N, cn - s0)
                    nc.tensor.matmul(
                        pt[:, s0 : s0 + sn],
                        lhsT,
                        cols[:, c0 + s0 : c0 + s0 + sn],
                        start=True,
                        stop=True,
                    )
                if use_dve:
                    dve_chunk(pt, cn, slot)
                else:
                    act_chunk(pt, 0, cn, slot)
                slot += 1

            nc.sync.dma_start(out=parts_d[:, :], in_=parts)
    nc.compile()
    return nc


def _prep_side(v):
    """v [N, D] fp32 -> (vh fp16 [N, D] = fp16(sqrt(A)*v), s fp64 [N]).

    s is computed from the EFFECTIVE (rounded) points so the diagonal
    exponent cancels exactly."""
    vh = (v.astype(np.float64) * A1).astype(np.float16)
    eff = vh.astype(np.float64) / A1
    s = -0.5 * np.sum(eff * eff, axis=1)
    return vh, s


def _g_hilo(g):
    """g fp64 -> (gh, gl) fp16 with gh*32768 + gl*32 ~= g."""
    gh = (g / 32768.0).astype(np.float16)
    res = g - gh.astype(np.float64) * 32768.0
    gl = (res / 32.0).astype(np.float16)
    return gh, gl


def _cols_tensor(vh, g):
    """[K, n] fp16 column tensor: [sqrt(A)*b; gh; gl]."""
    n = vh.shape[0]
    out = np.zeros((K, n), dtype=np.float16)
    out[:D] = vh.T
    out[D], out[D + 1] = _g_hilo(g)
    return np.ascontiguousarray(out)


def _rw_tensor(vh_block):
    """[K, PAD_BLOCK] fp16 row tensor: [sqrt(A)*a; 32768; 32]."""
    n = vh_block.shape[0]
    rw = np.zeros((K, PAD_BLOCK), dtype=np.float16)
    rw[:D, :n] = vh_block.T
    rw[D, :] = 32768.0
    rw[D + 1, :] = 32.0
    return rw


def _colsq_tensor(vh_block, g_block):
    """Coda columns: own block padded to PAD_BLOCK, pad cols killed."""
    n = vh_block.shape[0]
    vh_pad = np.zeros((PAD_BLOCK, D), dtype=np.float16)
    vh_pad[:n] = vh_block
    out = np.zeros((K, PAD_BLOCK), dtype=np.float16)
    out[:D] = vh_pad.T
    gh, gl = _g_hilo(g_block)
    out[D, :n], out[D + 1, :n] = gh, gl
    out[D, n:] = KILL_GH
    return out


def _make_in_maps(x, y):
    xh, sx = _prep_side(x)
    yh, sy = _prep_side(y)
    gx = A_SCALE * sx
    gy = A_SCALE * sy
    colsyf = _cols_tensor(yh, gy)

    in_maps = []
    factors = []  # per-core dict side_token -> [PAD_BLOCK] fp64 e^{s} (pads 0)
    for c in range(CORES):
        grp_a = c < 4
        blk = slice(BLOCK * c, BLOCK * (c + 1))
        ord1 = (np.arange(WIN1) + BLOCK * c) % N
        ord2 = (np.arange(WIN2) + BLOCK * c) % N
        if grp_a:
            cols1 = _cols_tensor(xh[ord1], gx[ord1] + DOUBLE)
            cols2 = _cols_tensor(yh[ord2], gy[ord2] + DOUBLE)
            rw1v, rw2v = xh[blk], yh[blk]
        else:
            cols1 = _cols_tensor(yh[ord1], gy[ord1] + DOUBLE)
            cols2 = _cols_tensor(xh[ord2], gx[ord2] + DOUBLE)
            rw1v, rw2v = yh[blk], xh[blk]
        in_maps.append(
            {
                "cols1": cols1,
                "cols2": cols2,
                "colsyf": colsyf,
                "colsqx": _colsq_tensor(xh[blk], gx[blk]),
                "colsqy": _colsq_tensor(yh[blk], gy[blk]),
                "rw1": _rw_tensor(rw1v),
                "rw2": _rw_tensor(rw2v),
                "rwx": _rw_tensor(xh[blk]),
                "rwy2": _rw_tensor(yh[blk]),
            }
        )
        fx = np.zeros(PAD_BLOCK)
        fx[:BLOCK] = np.exp(sx[blk])
        fy = np.zeros(PAD_BLOCK)
        fy[:BLOCK] = np.exp(sy[blk])
        factors.append(
            {
                "x": fx,
                "y": fy,
                "1": fx if grp_a else fy,
                "2": fy if grp_a else fx,
            }
        )
    return in_maps, factors


def kernel(x, y):
    from concourse.bass_utils import run_bass_kernel_spmd

    x = np.asarray(x, dtype=np.float32)
    y = np.asarray(y, dtype=np.float32)
    assert x.shape == (N, D) and y.shape == (N, D)

    if "nc" not in _CACHE:
        _CACHE["nc"] = _build_nc()
    nc = _CACHE["nc"]

    in_maps, factors = _make_in_maps(x, y)
    trace = os.environ.get("MMD_TRACE", "0") == "1"
    try:
        br = run_bass_kernel_spmd(
            nc, in_maps, core_ids=list(range(CORES)), trace=trace
        )
    except Exception:
        if not trace:
            raise
        import traceback

        traceback.print_exc()
        print("trace run failed; retrying without trace")
        br = run_bass_kernel_spmd(
            nc, in_maps, core_ids=list(range(CORES)), trace=False
        )
    _CACHE["last_results"] = br

    meta = _slot_meta()
    tot = np.zeros(2, dtype=np.float64)  # [same, cross]
    for core_res, fac in zip(br.results, factors):
        parts = core_res["parts"].astype(np.float64)  # [128, n_slots]
        for si, (acc, side, r) in enumerate(meta):
            f = fac[side][r * TILE : (r + 1) * TILE]
            tot[acc] += float(parts[:, si] @ f)
    val = tot[0] / (N * N) - 2.0 * tot[1] / (N * N)
    return np.array(val, dtype=np.float32)


# revision 11
# speedup vs baseline: 1.1857x; 1.1857x over previous
"""MMD loss (RBF kernel, sigma=1) on 8 Trainium2 NeuronCores.

kernel(x, y): x, y float32 [20000, 64] -> float32 scalar
    kxx/nX^2 + kyy/nY^2 - 2*kxy/(nX*nY),  k** = sum_ij exp(-||a_i-b_j||^2/2)

Design (v2)
-----------
exp(-(|a|^2+|b|^2-2ab)/2) = exp(a.b + s_b) * e^{s_a}, s_v = -|v|^2/2.
One fp16 matmul with K=66 produces the PRE-SCALED exponent
    m' = A*(a.b + s_b),  A = 2^23/ln2,
via rows [sqrt(A)*a (64); 32768; 32] x cols [sqrt(A)*b (64); gh; gl]
where gh*32768 + gl*32 = A*s_b (+2^23 for doubled columns: exp(m+ln2)=2e^m).
The per-row factor e^{s_a} is applied on the HOST to the per-row partial
sums (pad rows get factor 0), which keeps K = 66 and removes kill codes.

Each PSUM chunk is consumed by one of two engines:
  * ScalarE (ACT): exp activation (scale=1/A) + accum_out row sums.
  * VectorE (DVE): Schraudolph fast-exp: TS1 = rint(max(m'+B',0)) -> int32,
    bitcast int32 -> fp32 IS ~exp(m) (B' = 127*2^23 - 482573 tuned for
    zero-mean relative error); TS2 passthrough + accum_out row sums.
Both engines run concurrently; the PE (pinned cold at 1.2 GHz on this
platform, 1 col/cycle) becomes the bottleneck at ~98% busy.

Sharding: row blocks of 2500 across 8 cores (SPMD identical program).
kxx/kyy use block symmetry: every core computes two triangle-skipped
windows (12500 + 10000 wide, all columns doubled) plus 40 in-tile
diagonal squares (coda, weight 1); cores 0-3 take the x-side 12500
window (distance 1-4) + y-side 10000 (distance 1-3), cores 4-7 swap
sides. Since kxx and kyy carry the same coefficient, the host just sums
both into one "same-side" accumulator. kxy: full 20000 y columns.
"""

import os

import numpy as np

# problem dims (hardcoded per contract)
N = 20000
D = 64
CORES = 8
BLOCK = N // CORES  # 2500
TILE = 128
N_TILES = 20  # ceil(2500/128)
PAD_BLOCK = TILE * N_TILES  # 2560
WIN1 = 5 * BLOCK  # 12500
WIN2 = 4 * BLOCK  # 10000
K = D + 2  # 66 contraction rows
CHUNK = 2048  # chunk width (4 PSUM banks)
MM_N = 512  # matmul moving free dim (1 PSUM bank fp32)

A_SCALE = float(2**23) / float(np.log(2.0))  # 12102203.161561485
A1 = float(np.sqrt(A_SCALE))
B_SHIFT = float(127 * 2**23 - 482573)  # Schraudolph, zero-mean C
DOUBLE = float(2**23)  # A*ln2 exactly: doubles the term
KILL_GH = -65504.0  # pad cols: gh*32768 = -2.1e9 -> exp -> 0

_CACHE: dict = {}


def _patch_ldw_opt():
    """Enable walrus LDWEIGHTS dedup (--enable-ldw-opt): consecutive
    matmuls sharing a stationary operand skip the reload, recovering
    ~84ns/matmul of PE issue time."""
    if _CACHE.get("ldw_patched"):
        return
    import concourse.bass_utils as bu

    orig = bu.run_command

    def patched(cmd, *a, **kw):
        cmd = [
            c.replace("--enable-ldw-opt=false", "--enable-ldw-opt=true")
            if isinstance(c, str)
            else c
            for c in cmd
        ]
        return orig(cmd, *a, **kw)

    # NOTE: --enable-ldw-opt=true fails walrus codegen (visitInstLdweights
    # assert) on this compiler build; keep the flag off.
    _CACHE["ldw_patched"] = True


def _eq_chunks(total, chunk=CHUNK):
    """Equal-width EVEN chunks (each <= chunk); total must be even.
    Even widths keep the DVE decode pass in its 2x perf mode."""
    if total <= 0:
        return []
    assert total % 2 == 0
    n = -(-total // chunk)
    base2, rem2 = divmod(total // 2, n)
    out, pos = [], 0
    for i in range(n):
        w = 2 * (base2 + (1 if i < rem2 else 0))
        out.append((pos, w))
        pos += w
    return out


# items: (cols_name, rw_name, ncols, triangle?)
_ITEMS = [
    ("cols1", "rw1", WIN1, True),
    ("cols2", "rw2", WIN2, True),
    ("colsyf", "rwx", N, False),
]


def _chunk_schedule():
    """Main-loop chunks: list of (item_idx, r, c0, cn, use_dve)."""
    out = []
    gi = 0
    for it, (_c, _r, ncols, tri) in enumerate(_ITEMS):
        for r in range(N_TILES):
            base = TILE * (r + 1) if tri else 0
            for c0r, cn in _eq_chunks(ncols - base):
                use_dve = gi % 3 == 2
                out.append((it, r, base + c0r, cn, use_dve))
                gi += 1
    return out


def _slot_meta():
    """Per-slot (acc_class, side_token, r) for host reduction, in program
    order: 40 coda slots then the main-loop slots.

    side_token: which rw tensor's rows feed the slot's partitions:
      'x' -> rwx content, 'y' -> rwy2 content, '1' -> rw1, '2' -> rw2.
    acc_class: 0 = same-side (kxx+kyy), 1 = cross (kxy).
    """
    meta = []
    for r in range(N_TILES):
        meta.append((0, "x", r))
    for r in range(N_TILES):
        meta.append((0, "y", r))
    for it, r, _c0, _cn, _dve in _chunk_schedule():
        side = ("1", "2", "x")[it]
        acc = 1 if it == 2 else 0
        meta.append((acc, side, r))
    return meta


def _build_nc():
    import concourse.bacc as bacc
    import concourse.tile as tile
    from concourse import mybir

    f16 = mybir.dt.float16
    f32 = mybir.dt.float32
    i32 = mybir.dt.int32
    EXP = mybir.ActivationFunctionType.Exp
    ALU = mybir.AluOpType

    sched = _chunk_schedule()
    n_slots = 2 * N_TILES + len(sched)

    nc = bacc.Bacc("TRN2", target_bir_lowering=False)

    dram = {
        "cols1": nc.dram_tensor("cols1", [K, WIN1], f16, kind="ExternalInput"),
        "cols2": nc.dram_tensor("cols2", [K, WIN2], f16, kind="ExternalInput"),
        "colsyf": nc.dram_tensor("colsyf", [K, N], f16, kind="ExternalInput"),
        "colsqx": nc.dram_tensor("colsqx", [K, PAD_BLOCK], f16, kind="ExternalInput"),
        "colsqy": nc.dram_tensor("colsqy", [K, PAD_BLOCK], f16, kind="ExternalInput"),
        "rw1": nc.dram_tensor("rw1", [K, PAD_BLOCK], f16, kind="ExternalInput"),
        "rw2": nc.dram_tensor("rw2", [K, PAD_BLOCK], f16, kind="ExternalInput"),
        "rwx": nc.dram_tensor("rwx", [K, PAD_BLOCK], f16, kind="ExternalInput"),
        "rwy2": nc.dram_tensor("rwy2", [K, PAD_BLOCK], f16, kind="ExternalInput"),
    }
    parts_d = nc.dram_tensor("parts", [TILE, n_slots], f32, kind="ExternalOutput")

    with tile.TileContext(nc) as tc:
        with (
            tc.tile_pool(name="sb", bufs=1) as sb,
            tc.tile_pool(name="ps", bufs=2, space="PSUM") as ps,
        ):
            sbuf = {
                name: sb.tile([K, t.shape[1]], f16, name=f"sb_{name}")
                for name, t in dram.items()
            }
            parts = sb.tile([TILE, n_slots], f32)
            s_i32 = sb.tile([TILE, CHUNK], i32)
            s_dec = sb.tile([TILE, CHUNK], f32)
            zeros = sb.tile([TILE, 1], f32)
            nc.vector.memset(zeros, 0.0)
            nc.vector.memset(parts, 0.0)

            # DMA order: coda inputs first (they unblock the first chunks,
            # all on the fast sync queue), rw1 next (first main item), then
            # cols1 in small leading pieces so win1 r=0 unblocks early,
            # then the remaining windows round-robin on both queues.
            nc.sync.dma_start(out=sbuf["colsqx"], in_=dram["colsqx"][:, :])
            nc.sync.dma_start(out=sbuf["rwx"], in_=dram["rwx"][:, :])
            nc.sync.dma_start(out=sbuf["colsqy"], in_=dram["colsqy"][:, :])
            nc.sync.dma_start(out=sbuf["rwy2"], in_=dram["rwy2"][:, :])
            nc.sync.dma_start(out=sbuf["rw1"], in_=dram["rw1"][:, :])
            nc.gpsimd.dma_start(out=sbuf["rw2"], in_=dram["rw2"][:, :])
            for p0 in range(0, 8192, 4096):
                nc.gpsimd.dma_start(
                    out=sbuf["cols1"][:, p0 : p0 + 4096],
                    in_=dram["cols1"][:, p0 : p0 + 4096],
                )
            nc.sync.dma_start(
                out=sbuf["cols1"][:, 8192:WIN1], in_=dram["cols1"][:, 8192:WIN1]
            )
            dma_engines = [nc.gpsimd, nc.sync]
            ei = 0
            rest = [("cols2", 0, WIN2), ("colsyf", 0, N)]
            for name, start, total in rest:
                t = sbuf[name]
                left = total - start
                step = -(-left // 4)
                p0 = start
                while left > 0:
                    w = min(step, left)
                    dma_engines[ei % len(dma_engines)].dma_start(
                        out=t[:, p0 : p0 + w], in_=dram[name][:, p0 : p0 + w]
                    )
                    p0 += w
                    left -= w
                    ei += 1

            # fp32 warm-up burst: HAM un-throttle (K=8/8) fires stochastically
            # during dense matmul bursts; this costs ~7us of DMA-ramp shadow
            # and halves PE cycle time for the whole kernel when it hits.
            s_warm = sb.tile([TILE, 640], f16)
            nc.vector.memset(s_warm, 0.0)
            for i in range(8):
                ptw = ps.tile([TILE, CHUNK], f32, tag="pt", name=f"ptw{i}")
                for k in range(2):
                    nc.tensor.matmul(
                        ptw[:, k * MM_N : (k + 1) * MM_N],
                        s_warm[:, :TILE],
                        s_warm[:, TILE : TILE + MM_N],
                        start=True,
                        stop=True,
                    )

            slot = 0

            def act_chunk(pt, c0, cn, slot):
                nc.scalar.activation(
                    out=pt[:, c0 : c0 + cn],
                    in_=pt[:, c0 : c0 + cn],
                    func=EXP,
                    bias=zeros[:, 0:1],
                    scale=1.0 / A_SCALE,
                    accum_out=parts[:, slot : slot + 1],
                )

            def dve_chunk(pt, cn, slot):
                nc.vector.tensor_scalar(
                    out=s_i32[:, :cn],
                    in0=pt[:, :cn],
                    scalar1=B_SHIFT,
                    scalar2=0.0,
                    op0=ALU.add,
                    op1=ALU.max,
                )
                nc.vector.tensor_scalar(
                    out=s_dec[:, :cn],
                    in0=s_i32[:, :cn].bitcast(f32),
                    scalar1=1.0,
                    scalar2=0.0,
                    op0=ALU.mult,
                    op1=ALU.add,
                    accum_out=parts[:, slot : slot + 1],
                )

            # --- coda: 40 in-tile diagonal squares, one slot each ---
            coda = [("rwx", "colsqx", r) for r in range(N_TILES)] + [
                ("rwy2", "colsqy", r) for r in range(N_TILES)
            ]
            for g0 in range(0, 40, 16):
                grp = coda[g0 : g0 + 16]
                pt = ps.tile([TILE, CHUNK], f32, tag="pt", name=f"ptc{g0}")
                for k, (rwn, cqn, r) in enumerate(grp):
                    sl = slice(TILE * r, TILE * (r + 1))
                    nc.tensor.matmul(
                        pt[:, TILE * k : TILE * (k + 1)],
                        sbuf[rwn][:, sl],
                        sbuf[cqn][:, sl],
                        start=True,
                        stop=True,
                    )
                for k in range(len(grp)):
                    act_chunk(pt, TILE * k, TILE, slot)
                    slot += 1

            # --- main items ---
            for it, r, c0, cn, use_dve in _chunk_schedule():
                cols_name, rw_name, _ncols, _tri = _ITEMS[it]
                cols, rw = sbuf[cols_name], sbuf[rw_name]
                lhsT = rw[:, r * TILE : (r + 1) * TILE]
                pt = ps.tile([TILE, CHUNK], f32, tag="pt", name=f"pt{slot}")
                for s0 in range(0, cn, MM_N):
                    sn = min(MM_N, cn - s0)
                    nc.tensor.matmul(
                        pt[:, s0 : s0 + sn],
                        lhsT,
                        cols[:, c0 + s0 : c0 + s0 + sn],
                        start=True,
                        stop=True,
                    )
                if use_dve:
                    dve_chunk(pt, cn, slot)
                else:
                    act_chunk(pt, 0, cn, slot)
                slot += 1

            nc.sync.dma_start(out=parts_d[:, :], in_=parts)
    nc.compile()
    return nc


def _prep_side(v):
    """v [N, D] fp32 -> (vh fp16 [N, D] = fp16(sqrt(A)*v), s fp64 [N]).

    s is computed from the EFFECTIVE (rounded) points so the diagonal
    exponent cancels exactly."""
    vh = (v.astype(np.float64) * A1).astype(np.float16)
    eff = vh.astype(np.float64) / A1
    s = -0.5 * np.sum(eff * eff, axis=1)
    return vh, s


def _g_hilo(g):
    """g fp64 -> (gh, gl) fp16 with gh*32768 + gl*32 ~= g."""
    gh = (g / 32768.0).astype(np.float16)
    res = g - gh.astype(np.float64) * 32768.0
    gl = (res / 32.0).astype(np.float16)
    return gh, gl


def _cols_tensor(vh, g):
    """[K, n] fp16 column tensor: [sqrt(A)*b; gh; gl]."""
    n = vh.shape[0]
    out = np.zeros((K, n), dtype=np.float16)
    out[:D] = vh.T
    out[D], out[D + 1] = _g_hilo(g)
    return np.ascontiguousarray(out)


def _rw_tensor(vh_block):
    """[K, PAD_BLOCK] fp16 row tensor: [sqrt(A)*a; 32768; 32]."""
    n = vh_block.shape[0]
    rw = np.zeros((K, PAD_BLOCK), dtype=np.float16)
    rw[:D, :n] = vh_block.T
    rw[D, :] = 32768.0
    rw[D + 1, :] = 32.0
    return rw


def _colsq_tensor(vh_block, g_block):
    """Coda columns: own block padded to PAD_BLOCK, pad cols killed."""
    n = vh_block.shape[0]
    vh_pad = np.zeros((PAD_BLOCK, D), dtype=np.float16)
    vh_pad[:n] = vh_block
    out = np.zeros((K, PAD_BLOCK), dtype=np.float16)
    out[:D] = vh_pad.T
    gh, gl = _g_hilo(g_block)
    out[D, :n], out[D + 1, :n] = gh, gl
    out[D, n:] = KILL_GH
    return out


def _make_in_maps(x, y):
    xh, sx = _prep_side(x)
    yh, sy = _prep_side(y)
    gx = A_SCALE * sx
    gy = A_SCALE * sy
    colsyf = _cols_tensor(yh, gy)

    in_maps = []
    factors = []  # per-core dict side_token -> [PAD_BLOCK] fp64 e^{s} (pads 0)
    for c in range(CORES):
        grp_a = c < 4
        blk = slice(BLOCK * c, BLOCK * (c + 1))
        ord1 = (np.arange(WIN1) + BLOCK * c) % N
        ord2 = (np.arange(WIN2) + BLOCK * c) % N
        if grp_a:
            cols1 = _cols_tensor(xh[ord1], gx[ord1] + DOUBLE)
            cols2 = _cols_tensor(yh[ord2], gy[ord2] + DOUBLE)
            rw1v, rw2v = xh[blk], yh[blk]
        else:
            cols1 = _cols_tensor(yh[ord1], gy[ord1] + DOUBLE)
            cols2 = _cols_tensor(xh[ord2], gx[ord2] + DOUBLE)
            rw1v, rw2v = yh[blk], xh[blk]
        in_maps.append(
            {
                "cols1": cols1,
                "cols2": cols2,
                "colsyf": colsyf,
                "colsqx": _colsq_tensor(xh[blk], gx[blk]),
                "colsqy": _colsq_tensor(yh[blk], gy[blk]),
                "rw1": _rw_tensor(rw1v),
                "rw2": _rw_tensor(rw2v),
                "rwx": _rw_tensor(xh[blk]),
                "rwy2": _rw_tensor(yh[blk]),
            }
        )
        fx = np.zeros(PAD_BLOCK)
        fx[:BLOCK] = np.exp(sx[blk])
        fy = np.zeros(PAD_BLOCK)
        fy[:BLOCK] = np.exp(sy[blk])
        factors.append(
            {
                "x": fx,
                "y": fy,
                "1": fx if grp_a else fy,
                "2": fy if grp_a else fx,
            }
        )
    return in_maps, factors


def kernel(x, y):
    from concourse.bass_utils import run_bass_kernel_spmd

    x = np.asarray(x, dtype=np.float32)
    y = np.asarray(y, dtype=np.float32)
    assert x.shape == (N, D) and y.shape == (N, D)

    _patch_ldw_opt()
    if "nc" not in _CACHE:
        _CACHE["nc"] = _build_nc()
    nc = _CACHE["nc"]

    in_maps, factors = _make_in_maps(x, y)
    trace = os.environ.get("MMD_TRACE", "0") == "1"
    try:
        br = run_bass_kernel_spmd(
            nc, in_maps, core_ids=list(range(CORES)), trace=trace
        )
    except Exception:
        if not trace:
            raise
        import traceback

        traceback.print_exc()
        print("trace run failed; retrying without trace")
        br = run_bass_kernel_spmd(
            nc, in_maps, core_ids=list(range(CORES)), trace=False
        )
    _CACHE["last_results"] = br

    meta = _slot_meta()
    tot = np.zeros(2, dtype=np.float64)  # [same, cross]
    for core_res, fac in zip(br.results, factors):
        parts = core_res["parts"].astype(np.float64)  # [128, n_slots]
        for si, (acc, side, r) in enumerate(meta):
            f = fac[side][r * TILE : (r + 1) * TILE]
            tot[acc] += float(parts[:, si] @ f)
    val = tot[0] / (N * N) - 2.0 * tot[1] / (N * N)
    return np.array(val, dtype=np.float32)
